# revision 23
# baseline (speedup 1.0000x reference)
"""ArcFace inner-product kernel for one TRN2 chip (8 NeuronCores).

Problem: feat [4096, 512] f32, label [4096] i64, weights [20000, 512] f32.
  nf = l2norm(feat, axis=1); nw = l2norm(weights, axis=1)
  cos = nf @ nw.T                               [4096, 20000]
  ml  = 30 * cos(arccos(cos) + margin-at-label) [4096, 20000]
Returns (cos, ml).

Sharding: tensor-parallel over the class dim C. Each core receives the
full feat plus a 2500-row slice of weights and produces the matching
2500-column slices of both outputs. No collectives: the per-row label
fixup touches only 4096 of the 82M output elements and is applied on the
host after the column-slice gather.

v2 (vs the 543us f32 baseline):
  - bf16 everywhere the 2e-2 rel-err budget allows: matmul operands
    (nfT/nwT) and BOTH outputs are bf16 (host upcasts to f32).
    Measured rel-err ~2e-3 vs the f32 reference (10x margin). Output
    HBM traffic halves: 82MB -> 41MB per core, which was the roofline.
  - inputs are cast f32->bf16 during the load itself (SWDGE gpsimd
    DMA), freeing the two HWDGE rings for output strips.
  - output DMAs batched 4 m-strips per DMA (2.56MB bf16 per transfer,
    8 DMAs per output instead of 32).
  - PSUM: 6 matmul banks + 2 transpose banks.

Device kernel per core:
  1. weight tiles [128, 512]: cast-load, square+row-reduce (ACT Square
     + accum_out), rsqrt (DVE reciprocal + ACT sqrt), row-scale (ACT),
     PE-transpose into K-major bf16 SBUF tiles.
  2. feat tiles: same minus the row-scale - the feat norm is applied
     later as the per-partition ACT scale of the PSUM evict.
  3. For each output tile [128, n<=512]: 4 accumulating bf16 matmuls
     over K, evict cos = rf*psum (ACT -> bf16) and ml = 30*rf*psum
     (DVE -> bf16) into [128, 4*2500] strips; one DMA per strip group
     per output (sync ring: cos, scalar ring: ml).
"""

import math

import numpy as np

from concourse import bacc, mybir, tile
from concourse.masks import make_identity
from concourse.bass_utils import run_bass_kernel_spmd

B, D, C = 4096, 512, 20000
NCORES = 8
CLOC = C // NCORES  # 2500
KCH = D // 128      # 4 k-chunks
MT = B // 128       # 32 B-tiles
NT = (CLOC + 511) // 512  # 5 n-chunks (last = 452)
WT = (CLOC + 127) // 128  # 20 w-tiles (last = 68 rows)
SPD = 2             # m-strips batched per output DMA
GROUPS = MT // SPD  # 8 output DMA groups per output tensor
PF = 6              # feat-tile prefetch distance

SCALE = 30.0
MARGIN = 0.5
THRESH = -math.cos(MARGIN)
EXT_VAL = -MARGIN * math.sin(MARGIN)
COS_M = math.cos(MARGIN)
SIN_M = math.sin(MARGIN)

F32 = mybir.dt.float32
BF16 = mybir.dt.bfloat16

_NC_CACHE = {}


def _build_nc(repeats=1, timing=False):
    # timing=True: big outputs become Internal DRAM scratch (same DMA
    # traffic, nothing shipped over the axon tunnel per call) + a tiny
    # token ExternalOutput, fenced by reading the outputs back.
    nc = bacc.Bacc(
        "TRN2",
        target_bir_lowering=False,
        debug=False,
        num_devices=NCORES,
    )
    out_kind = "Internal" if timing else "ExternalOutput"
    feat = nc.dram_tensor("feat", [B, D], F32, kind="ExternalInput").ap()
    w = nc.dram_tensor("w", [CLOC, D], F32, kind="ExternalInput").ap()
    cos_o = nc.dram_tensor("cos_o", [B, CLOC], BF16, kind=out_kind).ap()
    ml_o = nc.dram_tensor("ml_o", [B, CLOC], BF16, kind=out_kind).ap()
    tok_o = (
        nc.dram_tensor("tok_o", [128, 4], F32, kind="ExternalOutput").ap()
        if timing
        else None
    )

    with tile.TileContext(nc) as tc:
        with (
            tc.tile_pool(name="const", bufs=1) as const_pool,
            tc.tile_pool(name="persist", bufs=1) as persist,
            tc.tile_pool(name="stage", bufs=6) as stage,
            tc.tile_pool(name="scratch", bufs=4) as scratch,
            tc.tile_pool(name="outs", bufs=3) as outs,
            tc.tile_pool(name="mm_psum", bufs=7, space="PSUM") as mm_psum,
            tc.tile_pool(name="tp_psum", bufs=1, space="PSUM") as tp_psum,
        ):
            ident = const_pool.tile([128, 128], BF16, tag="ident")
            make_identity(nc, ident[:])
            NSZ = [min(512, CLOC - n * 512) for n in range(NT)]

            def body(rep):
                sfx = f"_r{rep}" if rep else ""
                nfT = [
                    persist.tile([128, KCH * 128], BF16, tag=f"nfT{m}",
                                 name=f"nfT{m}{sfx}")
                    for m in range(MT)
                ]
                nwT = [
                    persist.tile([128, KCH * NSZ[n]], BF16, tag=f"nwT{n}",
                                 name=f"nwT{n}{sfx}")
                    for n in range(NT)
                ]
                rf = [
                    persist.tile([128, 1], F32, tag=f"rf{m}", name=f"rf{m}{sfx}")
                    for m in range(MT)
                ]
                emit(rep, nfT, nwT, rf)

            def rnorm(xt, rows, r_out):
                """r_out[p] = 1/||xt[p,:]|| for the first `rows` partitions.
                (ACT Square+accum_out; vector.tensor_tensor_reduce faults the
                hardware in this environment - do not use it.)"""
                sq = scratch.tile([128, D], F32, tag="sq")
                n2 = scratch.tile([128, 1], F32, tag="n2")
                nc.scalar.activation(
                    sq[:rows],
                    xt[:rows],
                    mybir.ActivationFunctionType.Square,
                    accum_out=n2[:rows],
                )
                rinv = scratch.tile([128, 1], F32, tag="rinv")
                nc.vector.reciprocal(rinv[:rows], n2[:rows])
                nc.scalar.sqrt(r_out[:rows], rinv[:rows])

            def transpose_blocks(xt, rows):
                """Transpose the 4 [rows,128] blocks of xt into one PSUM bank
                laid out k-major; returns the [128, KCH*128] PSUM tile."""
                tp = tp_psum.tile([128, KCH * 128], BF16, tag="tp")
                for k in range(KCH):
                    nc.tensor.transpose(
                        tp[:, k * 128 : k * 128 + rows],
                        xt[:rows, k * 128 : (k + 1) * 128],
                        ident[:rows, :rows],
                    )
                return tp

            def emit(rep, nfT, nwT, rf):
                sfx = f"_r{rep}" if rep else ""

                def wprep_tile(xt, t, rows):
                    """xt: [128(rows), D] bf16 view holding w tile t."""
                    n = t // 4
                    off = (t % 4) * 128
                    rw = scratch.tile([128, 1], F32, tag="rw")
                    rnorm(xt, rows, rw)
                    nc.vector.tensor_scalar_mul(xt[:rows], xt[:rows], rw[:rows])
                    tp = transpose_blocks(xt, rows)
                    src = tp[:].rearrange("p (k c) -> p k c", k=KCH)[:, :, :rows]
                    dst = nwT[n][:].rearrange("p (k c) -> p k c", k=KCH)[
                        :, :, off : off + rows
                    ]
                    nc.vector.tensor_copy(dst, src)

                def w_pair(p):
                    """Paired cast-loads (SWDGE fixed cost ~1us per DMA
                    dominates small loads, so batch 2 tiles per DMA)."""
                    xt2 = stage.tile([128, 2 * D], BF16, tag="xt2",
                                     name=f"xw{p}{sfx}")
                    nc.gpsimd.dma_start(
                        out=xt2[:].rearrange("p (s c) -> p s c", s=2),
                        in_=w[2 * p * 128 : (2 * p + 2) * 128, :].rearrange(
                            "(s p) c -> p s c", s=2
                        ),
                    )
                    wprep_tile(xt2[:, :D], 2 * p, 128)
                    wprep_tile(xt2[:, D:], 2 * p + 1, 128)

                def w_tail(t):  # ragged tail: 128 + 68 rows
                    rows = min(128, CLOC - t * 128)
                    xt2 = stage.tile([128, 2 * D], BF16, tag="xt2",
                                     name=f"xw_t{t}{sfx}")
                    nc.gpsimd.dma_start(
                        out=xt2[:rows, :D], in_=w[t * 128 : t * 128 + rows, :]
                    )
                    wprep_tile(xt2[:, :D], t, rows)

                # Feat prep: paired cast-loads; row norm saved per m-tile.
                def feat_pair(q):
                    xt2 = stage.tile([128, 2 * D], BF16, tag="xt2",
                                     name=f"xf{q}{sfx}")
                    nc.gpsimd.dma_start(
                        out=xt2[:].rearrange("p (s c) -> p s c", s=2),
                        in_=feat[2 * q * 128 : (2 * q + 2) * 128, :].rearrange(
                            "(s p) c -> p s c", s=2
                        ),
                    )
                    for b in (0, 1):
                        t = 2 * q + b
                        xt = xt2[:, b * D : (b + 1) * D]
                        rnorm(xt, 128, rf[t])
                        tp = transpose_blocks(xt, 128)
                        nc.vector.tensor_copy(nfT[t][:], tp[:])

                # Interleave the prefetched feat pairs into the w-prep
                # stream so both finish together and the main loop starts
                # without a pipeline refill lull.
                PFQ = 4  # feat prefetch distance, in pairs
                for p in range(WT // 2 - 1):
                    w_pair(p)
                    if p in (2, 4, 6, 8):
                        feat_pair((p - 2) // 2)
                w_tail(WT - 2)
                w_tail(WT - 1)

                for g in range(GROUPS):
                    cos_strip = outs.tile([128, SPD * CLOC], BF16,
                                          tag="cos_strip")
                    ml_strip = outs.tile([128, SPD * CLOC], BF16,
                                         tag="ml_strip")
                    for j in range(SPD):
                        m = g * SPD + j
                        if m % 2 == 0 and m // 2 + PFQ < MT // 2:
                            feat_pair(m // 2 + PFQ)
                        # k-outer: the stationary lhsT block is loaded once
                        # per (m, k) and reused across the 5 n-chunks
                        # (ldweights=False on the reuse matmuls), cutting
                        # PE-SEQ Ldweights dispatches 5x. Requires one live
                        # PSUM bank per n-chunk.
                        pss = [
                            mm_psum.tile([128, 512], F32, tag="mm",
                                         name=f"mm{m}_{n}{sfx}")
                            for n in range(NT)
                        ]
                        for k in range(KCH):
                            for n in range(NT):
                                nsz = NSZ[n]
                                inst = nc.tensor.matmul(
                                    pss[n][:, :nsz],
                                    lhsT=nfT[m][:, k * 128 : (k + 1) * 128],
                                    rhs=nwT[n][:, k * nsz : (k + 1) * nsz],
                                    start=(k == 0),
                                    stop=(k == KCH - 1),
                                )
                                if n > 0:
                                    inst.ldweights = False
                        # cos evicts: ACT n=0..3, DVE the short n=4 chunk
                        # (balances ACT ~130us vs DVE ~115us per body).
                        for n in range(NT):
                            nsz = NSZ[n]
                            c0 = j * CLOC + n * 512
                            if n < NT - 1:
                                nc.scalar.activation(
                                    cos_strip[:, c0 : c0 + nsz],
                                    pss[n][:, :nsz],
                                    mybir.ActivationFunctionType.Copy,
                                    scale=rf[m][:],
                                )
                            else:
                                nc.vector.tensor_scalar_mul(
                                    cos_strip[:, c0 : c0 + nsz],
                                    pss[n][:, :nsz],
                                    rf[m][:],
                                )
                        # ml = 30*cos read from the bf16 strip: all-bf16
                        # SBUF operands put DVE in its 2x/4x 16-bit mode,
                        # and it is one instruction per m instead of five.
                        nc.vector.tensor_scalar_mul(
                            ml_strip[:, j * CLOC : (j + 1) * CLOC],
                            cos_strip[:, j * CLOC : (j + 1) * CLOC],
                            SCALE,
                        )
                    r0 = g * SPD * 128
                    nc.sync.dma_start(
                        out=cos_o[r0 : r0 + SPD * 128, :].rearrange(
                            "(s p) c -> p s c", s=SPD
                        ),
                        in_=cos_strip[:].rearrange("p (s c) -> p s c", s=SPD),
                    )
                    nc.scalar.dma_start(
                        out=ml_o[r0 : r0 + SPD * 128, :].rearrange(
                            "(s p) c -> p s c", s=SPD
                        ),
                        in_=ml_strip[:].rearrange("p (s c) -> p s c", s=SPD),
                    )

            for rep in range(repeats):
                body(rep)

            if timing:
                # Fence: read back a sliver of each Internal output on both
                # output rings (FIFO per ring), keeping writes live vs
                # dead-store elimination and gating the token on the drain.
                tok = const_pool.tile([128, 4], F32, tag="tok")
                tokb = const_pool.tile([128, 4], BF16, tag="tokb")
                nc.sync.dma_start(out=tokb[:, :2], in_=cos_o[B - 128 :, :2])
                nc.scalar.dma_start(out=tokb[:, 2:4], in_=ml_o[B - 128 :, :2])
                nc.vector.tensor_copy(tok[:], tokb[:])
                nc.sync.dma_start(out=tok_o, in_=tok[:])

    nc.compile()
    return nc


def _purge_neff_cache():
    """The neuronxcc NEFF cache key does NOT cover the embedded BIR
    payload (verified: edited kernels cache-hit stale NEFFs compiled
    from different BIR). Purge it so this process always executes the
    NEFF compiled from THIS module."""
    import shutil

    shutil.rmtree("/root/.neuron-compile-cache", ignore_errors=True)


def _get_nc():
    if "nc" not in _NC_CACHE:
        _purge_neff_cache()
        _NC_CACHE["nc"] = _build_nc()
    return _NC_CACHE["nc"]


def make_in_maps(feat, weights):
    feat = np.ascontiguousarray(np.asarray(feat, dtype=np.float32))
    weights = np.ascontiguousarray(np.asarray(weights, dtype=np.float32))
    return [
        {"feat": feat, "w": weights[k * CLOC : (k + 1) * CLOC]}
        for k in range(NCORES)
    ]


def assemble(results, label):
    """Gather per-core column slices (bf16 -> f32) and apply the per-row
    label fixup."""
    cos = np.empty((B, C), np.float32)
    ml = np.empty((B, C), np.float32)
    for k in range(NCORES):
        cos[:, k * CLOC : (k + 1) * CLOC] = results[k]["cos_o"].astype(
            np.float32
        )
        ml[:, k * CLOC : (k + 1) * CLOC] = results[k]["ml_o"].astype(
            np.float32
        )
    idx = np.arange(B)
    lab = np.asarray(label).astype(np.int64)
    cil = cos[idx, lab]
    sin_il = np.sqrt(np.maximum(0.0, 1.0 - cil * cil)).astype(np.float32)
    hit = cil > THRESH
    ml[idx, lab] = np.where(
        hit,
        SCALE * (cil * COS_M - sin_il * SIN_M),
        SCALE * (cil + EXT_VAL),
    ).astype(np.float32)
    return cos, ml


def kernel(feat, label, weights):
    nc = _get_nc()
    in_maps = make_in_maps(feat, weights)
    res = run_bass_kernel_spmd(nc, in_maps, core_ids=list(range(NCORES)))
    return assemble(res.results, label)


# revision 24
# speedup vs baseline: 1.0028x; 1.0028x over previous
"""ArcFace inner-product kernel for one TRN2 chip (8 NeuronCores).

Problem: feat [4096, 512] f32, label [4096] i64, weights [20000, 512] f32.
  nf = l2norm(feat, axis=1); nw = l2norm(weights, axis=1)
  cos = nf @ nw.T                               [4096, 20000]
  ml  = 30 * cos(arccos(cos) + margin-at-label) [4096, 20000]
Returns (cos, ml).

Sharding: tensor-parallel over the class dim C. Each core receives the
full feat plus a 2500-row slice of weights and produces the matching
2500-column slices of both outputs. No collectives: the per-row label
fixup touches only 4096 of the 82M output elements and is applied on the
host after the column-slice gather.

v2 (vs the 543us f32 baseline):
  - bf16 everywhere the 2e-2 rel-err budget allows: matmul operands
    (nfT/nwT) and BOTH outputs are bf16 (host upcasts to f32).
    Measured rel-err ~2e-3 vs the f32 reference (10x margin). Output
    HBM traffic halves: 82MB -> 41MB per core, which was the roofline.
  - inputs are cast f32->bf16 during the load itself (SWDGE gpsimd
    DMA), freeing the two HWDGE rings for output strips.
  - output DMAs batched 4 m-strips per DMA (2.56MB bf16 per transfer,
    8 DMAs per output instead of 32).
  - PSUM: 6 matmul banks + 2 transpose banks.

Device kernel per core:
  1. weight tiles [128, 512]: cast-load, square+row-reduce (ACT Square
     + accum_out), rsqrt (DVE reciprocal + ACT sqrt), row-scale (ACT),
     PE-transpose into K-major bf16 SBUF tiles.
  2. feat tiles: same minus the row-scale - the feat norm is applied
     later as the per-partition ACT scale of the PSUM evict.
  3. For each output tile [128, n<=512]: 4 accumulating bf16 matmuls
     over K, evict cos = rf*psum (ACT -> bf16) and ml = 30*rf*psum
     (DVE -> bf16) into [128, 4*2500] strips; one DMA per strip group
     per output (sync ring: cos, scalar ring: ml).
"""

import math

import numpy as np

from concourse import bacc, mybir, tile
from concourse.masks import make_identity
from concourse.bass_utils import run_bass_kernel_spmd

B, D, C = 4096, 512, 20000
NCORES = 8
CLOC = C // NCORES  # 2500
KCH = D // 128      # 4 k-chunks
MT = B // 128       # 32 B-tiles
NT = (CLOC + 511) // 512  # 5 n-chunks (last = 452)
WT = (CLOC + 127) // 128  # 20 w-tiles (last = 68 rows)
SPD = 2             # m-strips batched per output DMA
GROUPS = MT // SPD  # 8 output DMA groups per output tensor
PF = 6              # feat-tile prefetch distance

SCALE = 30.0
MARGIN = 0.5
THRESH = -math.cos(MARGIN)
EXT_VAL = -MARGIN * math.sin(MARGIN)
COS_M = math.cos(MARGIN)
SIN_M = math.sin(MARGIN)

F32 = mybir.dt.float32
BF16 = mybir.dt.bfloat16

_NC_CACHE = {}


def _build_nc(repeats=1, timing=False):
    # timing=True: big outputs become Internal DRAM scratch (same DMA
    # traffic, nothing shipped over the axon tunnel per call) + a tiny
    # token ExternalOutput, fenced by reading the outputs back.
    nc = bacc.Bacc(
        "TRN2",
        target_bir_lowering=False,
        debug=False,
        num_devices=NCORES,
    )
    out_kind = "Internal" if timing else "ExternalOutput"
    feat = nc.dram_tensor("feat", [B, D], F32, kind="ExternalInput").ap()
    w = nc.dram_tensor("w", [CLOC, D], F32, kind="ExternalInput").ap()
    cos_o = nc.dram_tensor("cos_o", [B, CLOC], BF16, kind=out_kind).ap()
    ml_o = nc.dram_tensor("ml_o", [B, CLOC], BF16, kind=out_kind).ap()
    tok_o = (
        nc.dram_tensor("tok_o", [128, 4], F32, kind="ExternalOutput").ap()
        if timing
        else None
    )

    with tile.TileContext(nc) as tc:
        with (
            tc.tile_pool(name="const", bufs=1) as const_pool,
            tc.tile_pool(name="persist", bufs=1) as persist,
            tc.tile_pool(name="stage", bufs=8) as stage,
            tc.tile_pool(name="scratch", bufs=4) as scratch,
            tc.tile_pool(name="outs", bufs=3) as outs,
            tc.tile_pool(name="mm_psum", bufs=7, space="PSUM") as mm_psum,
            tc.tile_pool(name="tp_psum", bufs=1, space="PSUM") as tp_psum,
        ):
            ident = const_pool.tile([128, 128], BF16, tag="ident")
            make_identity(nc, ident[:])
            NSZ = [min(512, CLOC - n * 512) for n in range(NT)]

            def body(rep):
                sfx = f"_r{rep}" if rep else ""
                nfT = [
                    persist.tile([128, KCH * 128], BF16, tag=f"nfT{m}",
                                 name=f"nfT{m}{sfx}")
                    for m in range(MT)
                ]
                nwT = [
                    persist.tile([128, KCH * NSZ[n]], BF16, tag=f"nwT{n}",
                                 name=f"nwT{n}{sfx}")
                    for n in range(NT)
                ]
                rf = [
                    persist.tile([128, 1], F32, tag=f"rf{m}", name=f"rf{m}{sfx}")
                    for m in range(MT)
                ]
                emit(rep, nfT, nwT, rf)

            def rnorm(xt, rows, r_out):
                """r_out[p] = 1/||xt[p,:]|| for the first `rows` partitions.
                (ACT Square+accum_out; vector.tensor_tensor_reduce faults the
                hardware in this environment - do not use it.)"""
                sq = scratch.tile([128, D], F32, tag="sq")
                n2 = scratch.tile([128, 1], F32, tag="n2")
                nc.scalar.activation(
                    sq[:rows],
                    xt[:rows],
                    mybir.ActivationFunctionType.Square,
                    accum_out=n2[:rows],
                )
                rinv = scratch.tile([128, 1], F32, tag="rinv")
                nc.vector.reciprocal(rinv[:rows], n2[:rows])
                nc.scalar.sqrt(r_out[:rows], rinv[:rows])

            def transpose_blocks(xt, rows):
                """Transpose the 4 [rows,128] blocks of xt into one PSUM bank
                laid out k-major; returns the [128, KCH*128] PSUM tile."""
                tp = tp_psum.tile([128, KCH * 128], BF16, tag="tp")
                for k in range(KCH):
                    nc.tensor.transpose(
                        tp[:, k * 128 : k * 128 + rows],
                        xt[:rows, k * 128 : (k + 1) * 128],
                        ident[:rows, :rows],
                    )
                return tp

            def emit(rep, nfT, nwT, rf):
                sfx = f"_r{rep}" if rep else ""

                def wprep_tile(xt, t, rows):
                    """xt: [128(rows), D] bf16 view holding w tile t."""
                    n = t // 4
                    off = (t % 4) * 128
                    rw = scratch.tile([128, 1], F32, tag="rw")
                    rnorm(xt, rows, rw)
                    nc.vector.tensor_scalar_mul(xt[:rows], xt[:rows], rw[:rows])
                    tp = transpose_blocks(xt, rows)
                    src = tp[:].rearrange("p (k c) -> p k c", k=KCH)[:, :, :rows]
                    dst = nwT[n][:].rearrange("p (k c) -> p k c", k=KCH)[
                        :, :, off : off + rows
                    ]
                    nc.vector.tensor_copy(dst, src)

                def w_pair(p):
                    """Paired cast-loads (SWDGE fixed cost ~1us per DMA
                    dominates small loads, so batch 2 tiles per DMA)."""
                    xt2 = stage.tile([128, 2 * D], BF16, tag="xt2",
                                     name=f"xw{p}{sfx}")
                    nc.gpsimd.dma_start(
                        out=xt2[:].rearrange("p (s c) -> p s c", s=2),
                        in_=w[2 * p * 128 : (2 * p + 2) * 128, :].rearrange(
                            "(s p) c -> p s c", s=2
                        ),
                    )
                    wprep_tile(xt2[:, :D], 2 * p, 128)
                    wprep_tile(xt2[:, D:], 2 * p + 1, 128)

                def w_tail(t):  # ragged tail: 128 + 68 rows
                    rows = min(128, CLOC - t * 128)
                    xt2 = stage.tile([128, 2 * D], BF16, tag="xt2",
                                     name=f"xw_t{t}{sfx}")
                    nc.gpsimd.dma_start(
                        out=xt2[:rows, :D], in_=w[t * 128 : t * 128 + rows, :]
                    )
                    wprep_tile(xt2[:, :D], t, rows)

                # Feat prep: paired cast-loads; row norm saved per m-tile.
                def feat_pair(q):
                    xt2 = stage.tile([128, 2 * D], BF16, tag="xt2",
                                     name=f"xf{q}{sfx}")
                    nc.gpsimd.dma_start(
                        out=xt2[:].rearrange("p (s c) -> p s c", s=2),
                        in_=feat[2 * q * 128 : (2 * q + 2) * 128, :].rearrange(
                            "(s p) c -> p s c", s=2
                        ),
                    )
                    for b in (0, 1):
                        t = 2 * q + b
                        xt = xt2[:, b * D : (b + 1) * D]
                        rnorm(xt, 128, rf[t])
                        tp = transpose_blocks(xt, 128)
                        nc.vector.tensor_copy(nfT[t][:], tp[:])

                # Interleave the prefetched feat pairs into the w-prep
                # stream so both finish together and the main loop starts
                # without a pipeline refill lull.
                PFQ = 3  # feat prefetch distance, in pairs
                for p in range(WT // 2 - 1):
                    w_pair(p)
                    if p in (2, 4, 6):
                        feat_pair((p - 2) // 2)
                w_tail(WT - 2)
                w_tail(WT - 1)

                for g in range(GROUPS):
                    cos_strip = outs.tile([128, SPD * CLOC], BF16,
                                          tag="cos_strip")
                    ml_strip = outs.tile([128, SPD * CLOC], BF16,
                                         tag="ml_strip")
                    for j in range(SPD):
                        m = g * SPD + j
                        if m % 2 == 0 and m // 2 + PFQ < MT // 2:
                            feat_pair(m // 2 + PFQ)
                        # k-outer: the stationary lhsT block is loaded once
                        # per (m, k) and reused across the 5 n-chunks
                        # (ldweights=False on the reuse matmuls), cutting
                        # PE-SEQ Ldweights dispatches 5x. Requires one live
                        # PSUM bank per n-chunk.
                        pss = [
                            mm_psum.tile([128, 512], F32, tag="mm",
                                         name=f"mm{m}_{n}{sfx}")
                            for n in range(NT)
                        ]
                        for k in range(KCH):
                            for n in range(NT):
                                nsz = NSZ[n]
                                inst = nc.tensor.matmul(
                                    pss[n][:, :nsz],
                                    lhsT=nfT[m][:, k * 128 : (k + 1) * 128],
                                    rhs=nwT[n][:, k * nsz : (k + 1) * nsz],
                                    start=(k == 0),
                                    stop=(k == KCH - 1),
                                )
                                if n > 0:
                                    inst.ldweights = False
                        # cos evicts: ACT n=0..3, DVE the short n=4 chunk
                        # (balances ACT ~130us vs DVE ~115us per body).
                        for n in range(NT):
                            nsz = NSZ[n]
                            c0 = j * CLOC + n * 512
                            if n < NT - 1:
                                nc.scalar.activation(
                                    cos_strip[:, c0 : c0 + nsz],
                                    pss[n][:, :nsz],
                                    mybir.ActivationFunctionType.Copy,
                                    scale=rf[m][:],
                                )
                            else:
                                nc.vector.tensor_scalar_mul(
                                    cos_strip[:, c0 : c0 + nsz],
                                    pss[n][:, :nsz],
                                    rf[m][:],
                                )
                        # ml = 30*cos read from the bf16 strip: all-bf16
                        # SBUF operands put DVE in its 2x/4x 16-bit mode,
                        # and it is one instruction per m instead of five.
                        nc.vector.tensor_scalar_mul(
                            ml_strip[:, j * CLOC : (j + 1) * CLOC],
                            cos_strip[:, j * CLOC : (j + 1) * CLOC],
                            SCALE,
                        )
                    r0 = g * SPD * 128
                    nc.sync.dma_start(
                        out=cos_o[r0 : r0 + SPD * 128, :].rearrange(
                            "(s p) c -> p s c", s=SPD
                        ),
                        in_=cos_strip[:].rearrange("p (s c) -> p s c", s=SPD),
                    )
                    nc.scalar.dma_start(
                        out=ml_o[r0 : r0 + SPD * 128, :].rearrange(
                            "(s p) c -> p s c", s=SPD
                        ),
                        in_=ml_strip[:].rearrange("p (s c) -> p s c", s=SPD),
                    )

            for rep in range(repeats):
                body(rep)

            if timing:
                # Fence: read back a sliver of each Internal output on both
                # output rings (FIFO per ring), keeping writes live vs
                # dead-store elimination and gating the token on the drain.
                tok = const_pool.tile([128, 4], F32, tag="tok")
                tokb = const_pool.tile([128, 4], BF16, tag="tokb")
                nc.sync.dma_start(out=tokb[:, :2], in_=cos_o[B - 128 :, :2])
                nc.scalar.dma_start(out=tokb[:, 2:4], in_=ml_o[B - 128 :, :2])
                nc.vector.tensor_copy(tok[:], tokb[:])
                nc.sync.dma_start(out=tok_o, in_=tok[:])

    nc.compile()
    return nc


def _purge_neff_cache():
    """The neuronxcc NEFF cache key does NOT cover the embedded BIR
    payload (verified: edited kernels cache-hit stale NEFFs compiled
    from different BIR). Purge it so this process always executes the
    NEFF compiled from THIS module."""
    import shutil

    shutil.rmtree("/root/.neuron-compile-cache", ignore_errors=True)


def _get_nc():
    if "nc" not in _NC_CACHE:
        _purge_neff_cache()
        _NC_CACHE["nc"] = _build_nc()
    return _NC_CACHE["nc"]


def make_in_maps(feat, weights):
    feat = np.ascontiguousarray(np.asarray(feat, dtype=np.float32))
    weights = np.ascontiguousarray(np.asarray(weights, dtype=np.float32))
    return [
        {"feat": feat, "w": weights[k * CLOC : (k + 1) * CLOC]}
        for k in range(NCORES)
    ]


def assemble(results, label):
    """Gather per-core column slices (bf16 -> f32) and apply the per-row
    label fixup."""
    cos = np.empty((B, C), np.float32)
    ml = np.empty((B, C), np.float32)
    for k in range(NCORES):
        cos[:, k * CLOC : (k + 1) * CLOC] = results[k]["cos_o"].astype(
            np.float32
        )
        ml[:, k * CLOC : (k + 1) * CLOC] = results[k]["ml_o"].astype(
            np.float32
        )
    idx = np.arange(B)
    lab = np.asarray(label).astype(np.int64)
    cil = cos[idx, lab]
    sin_il = np.sqrt(np.maximum(0.0, 1.0 - cil * cil)).astype(np.float32)
    hit = cil > THRESH
    ml[idx, lab] = np.where(
        hit,
        SCALE * (cil * COS_M - sin_il * SIN_M),
        SCALE * (cil + EXT_VAL),
    ).astype(np.float32)
    return cos, ml


def kernel(feat, label, weights):
    nc = _get_nc()
    in_maps = make_in_maps(feat, weights)
    res = run_bass_kernel_spmd(nc, in_maps, core_ids=list(range(NCORES)))
    return assemble(res.results, label)
